# revision 7
# baseline (speedup 1.0000x reference)
"""KSparse top-k row masking on 8 trn2 NeuronCores.

Algorithm per 128-row tile (rows on partitions, 8192 elements on free dim):
  1. P counting passes on the Scalar engine: activation(Sign, bias=-t) with
     accum_out gives  s = #gt - #le ;  c = (s + n)/2  exactly.  Per-row Newton
     updates with damped fixed density; a row freezes its threshold once its
     count lands in the window [k-31, k].
  2. Extraction on the Vector engine: y = (x is_le t_f) * x keeps the excluded
     elements (values preserved); 64 seg-max8 ops build a candidate array C
     [128, 512] that provably contains the top-32 excluded values; 4 rounds of
     (max8 + match_replace) yield G [128, 32] = sorted top-32 excluded.
     v* = G[k - c_f]  ==  (k+1)-th largest of the row, bit-exact.
  3. Apply: out = (x is_gt v*) * x   (bit-identical to the reference mask-mul).

Data parallel across 8 cores on the batch axis (512 rows/core), no comms.
"""
import math
import sys

sys.path.insert(0, "/opt/trn_rl_repo")

import numpy as np

import concourse.bacc as bacc
import concourse.mybir as mybir
import concourse.tile as tile
from concourse import bass_utils

dt = mybir.dt
Alu = mybir.AluOpType
Act = mybir.ActivationFunctionType

N_CORES = 8
ROWS_PER_CORE = 512
N = 8192
N_TILES = ROWS_PER_CORE // 128

DAMPS = (1.0, 0.8, 0.6, 0.45)   # P = len+1 counting passes
WIN = 32                         # extraction window (count in [k-31, k])
SEG = 64                         # segments for seg-max8 (128 elements each)


def _norm_ppf(p):
    """Inverse standard normal CDF via bisection on erfc (no scipy)."""
    lo, hi = -10.0, 10.0
    for _ in range(200):
        mid = 0.5 * (lo + hi)
        if 0.5 * math.erfc(-mid / math.sqrt(2.0)) < p:
            lo = mid
        else:
            hi = mid
    return 0.5 * (lo + hi)


def _build(k):
    n = N
    t_target = k - (WIN - 1) / 2.0                  # aim count at window center
    t0 = _norm_ppf(1.0 - t_target / n)
    dens = n * math.exp(-0.5 * t0 * t0) / math.sqrt(2.0 * math.pi)
    c_off = n / 2.0 - t_target                      # d = s*0.5 + c_off  (= c - T)
    r_off = (WIN - 1) / 2.0                         # r = k - c = r_off - d
    half_win = WIN / 2.0 - 0.25                     # |d| <= 15.5 (strict margin)

    nc = bacc.Bacc("TRN2", target_bir_lowering=False, debug=False,
                   num_devices=N_CORES)
    x_dram = nc.dram_tensor("x", [ROWS_PER_CORE, N], dt.float32,
                            kind="ExternalInput").ap()
    iota_dram = nc.dram_tensor("iota", [128, WIN], dt.float32,
                               kind="ExternalInput").ap()
    out_dram = nc.dram_tensor("out", [ROWS_PER_CORE, N], dt.float32,
                              kind="ExternalOutput").ap()

    with tile.TileContext(nc) as tc:
        with (
            tc.tile_pool(name="xp", bufs=N_TILES) as xp,
            tc.tile_pool(name="yp", bufs=2) as yp,
            tc.tile_pool(name="scr", bufs=1) as scrp,
            tc.tile_pool(name="small", bufs=1) as sp,
            tc.tile_pool(name="cand", bufs=2) as cp,
        ):
            iota32 = sp.tile([128, WIN], dt.float32)
            nc.sync.dma_start(iota32[:], iota_dram[:])

            sgn_scratch = scrp.tile([128, N], dt.float8e4)

            xts = []
            for i in range(N_TILES):
                xt = xp.tile([128, N], dt.float32, tag="x")
                nc.sync.dma_start(xt[:], x_dram[i * 128:(i + 1) * 128, :])
                xts.append(xt)

            for i in range(N_TILES):
                xt = xts[i]
                # --- per-tile iteration state [128,1]
                negt = sp.tile([128, 1], dt.float32, tag=f"negt{i}")
                nb = sp.tile([128, 1], dt.float32, tag=f"nb{i}")
                db = sp.tile([128, 1], dt.float32, tag=f"db{i}")
                hg = sp.tile([128, 1], dt.float32, tag=f"hg{i}")
                nc.vector.memset(negt[:], -t0)
                nc.vector.memset(nb[:], -t0)   # negt_best (fallback = current)
                nc.vector.memset(db[:], 0.0)
                nc.vector.memset(hg[:], 0.0)

                s_acc = sp.tile([128, 1], dt.float32, tag=f"s{i}")
                d = sp.tile([128, 1], dt.float32, tag=f"d{i}")
                w = sp.tile([128, 1], dt.float32, tag=f"w{i}")
                wb = sp.tile([128, 1], dt.float32, tag=f"wb{i}")
                good = sp.tile([128, 1], dt.float32, tag=f"g{i}")
                tmp = sp.tile([128, 1], dt.float32, tag=f"tmp{i}")
                nu = sp.tile([128, 1], dt.float32, tag=f"nu{i}")

                n_pass = len(DAMPS) + 1
                for p in range(n_pass):
                    # count: s = sum sign(x - t)
                    nc.scalar.activation(sgn_scratch[:], xt[:], Act.Sign,
                                         bias=negt[:], scale=1.0,
                                         accum_out=s_acc[:])
                    # d = c - T = s*0.5 + c_off   (two single-scalar ops:
                    # dual-scalar tensor_scalar drops op1 on this path)
                    nc.vector.tensor_scalar(d[:], s_acc[:], 0.5, None, Alu.mult)
                    nc.vector.tensor_scalar(d[:], d[:], c_off, None, Alu.add)
                    # w = (d >= -half_win) * (d <= half_win)
                    nc.vector.tensor_scalar(w[:], d[:], -half_win, None,
                                            Alu.is_ge)
                    nc.vector.tensor_scalar(wb[:], d[:], half_win, None,
                                            Alu.is_le)
                    nc.vector.tensor_tensor(w[:], w[:], wb[:], Alu.mult)
                    # good = w AND NOT hg  (both 0/1)  = (w is_gt hg)
                    nc.vector.tensor_tensor(good[:], w[:], hg[:], Alu.is_gt)
                    # hg |= w
                    nc.vector.tensor_tensor(hg[:], hg[:], w[:], Alu.max)
                    # nb += good*(negt - nb);  db += good*(d - db)
                    nc.vector.tensor_tensor(tmp[:], negt[:], nb[:], Alu.subtract)
                    nc.vector.scalar_tensor_tensor(nb[:], tmp[:], good[:], nb[:],
                                                   Alu.mult, Alu.add)
                    nc.vector.tensor_tensor(tmp[:], d[:], db[:], Alu.subtract)
                    nc.vector.scalar_tensor_tensor(db[:], tmp[:], good[:], db[:],
                                                   Alu.mult, Alu.add)
                    if p < n_pass - 1:
                        # negt += (hg - 1) * d * damp/dens
                        #   (nu = hg - 1 in {-1, 0};  Kp = damp/dens > 0;
                        #    negt' = negt + nu*d*Kp  ==  t' = t + (1-hg)*d*Kp)
                        kp = DAMPS[p] / dens
                        nc.vector.tensor_scalar(nu[:], hg[:], 1.0, None,
                                                Alu.subtract)
                        nc.vector.tensor_scalar(tmp[:], d[:], kp, None, Alu.mult)
                        nc.vector.scalar_tensor_tensor(negt[:], tmp[:], nu[:],
                                                       negt[:], Alu.mult, Alu.add)

                # fallback for never-frozen rows: use last (negt, d)
                nc.vector.tensor_tensor(tmp[:], negt[:], nb[:], Alu.subtract)
                nc.vector.tensor_scalar(nu[:], hg[:], 1.0, None, Alu.subtract)
                # nb += (1-hg)*(negt-nb)  ->  nb -= nu*tmp
                nc.vector.tensor_scalar(tmp[:], tmp[:], -1.0, None, Alu.mult)
                nc.vector.scalar_tensor_tensor(nb[:], tmp[:], nu[:], nb[:],
                                               Alu.mult, Alu.add)
                nc.vector.tensor_tensor(tmp[:], d[:], db[:], Alu.subtract)
                nc.vector.tensor_scalar(tmp[:], tmp[:], -1.0, None, Alu.mult)
                nc.vector.scalar_tensor_tensor(db[:], tmp[:], nu[:], db[:],
                                               Alu.mult, Alu.add)

                tpos = sp.tile([128, 1], dt.float32, tag=f"tp{i}")
                nc.vector.tensor_scalar(tpos[:], nb[:], -1.0, None, Alu.mult)

                # --- extraction
                y = yp.tile([128, N], dt.float32, tag="y")
                nc.vector.scalar_tensor_tensor(y[:], xt[:], tpos[:], xt[:],
                                               Alu.is_le, Alu.mult)
                cand = cp.tile([128, SEG * 8], dt.float32, tag="c")
                segw = N // SEG
                for s in range(SEG):
                    nc.vector.max(out=cand[:, s * 8:(s + 1) * 8],
                                  in_=y[:, s * segw:(s + 1) * segw])
                G = sp.tile([128, WIN], dt.float32, tag=f"G{i}")
                rounds = WIN // 8
                for j in range(rounds):
                    nc.vector.max(out=G[:, j * 8:(j + 1) * 8], in_=cand[:])
                    if j < rounds - 1:
                        cand2 = cp.tile([128, SEG * 8], dt.float32, tag="c")
                        nc.vector.match_replace(out=cand2[:],
                                                in_to_replace=G[:, j * 8:(j + 1) * 8],
                                                in_values=cand[:],
                                                imm_value=-1e30)
                        cand = cand2

                # r = r_off - d_f   in [0, WIN-1], clamped
                r = sp.tile([128, 1], dt.float32, tag=f"r{i}")
                nc.vector.tensor_scalar(r[:], db[:], -1.0, r_off,
                                        Alu.mult, Alu.add)
                nc.vector.tensor_scalar(r[:], r[:], 0.0, float(WIN - 1),
                                        Alu.max, Alu.min)
                oh = sp.tile([128, WIN], dt.float32, tag=f"oh{i}")
                nc.vector.tensor_scalar(oh[:], iota32[:], r[:], None,
                                        Alu.is_equal)
                trash = sp.tile([128, WIN], dt.float32, tag=f"tr{i}")
                vhat = sp.tile([128, 1], dt.float32, tag=f"v{i}")
                nc.vector.scalar_tensor_tensor(trash[:], oh[:], 1.0, G[:],
                                               Alu.mult, Alu.mult,
                                               accum_out=vhat[:])

                # --- apply (write over y slot) + store
                nc.vector.scalar_tensor_tensor(y[:], xt[:], vhat[:], xt[:],
                                               Alu.is_gt, Alu.mult)
                nc.sync.dma_start(out_dram[i * 128:(i + 1) * 128, :], y[:])
    nc.compile()
    return nc


_cache = {}


def _get(k):
    if k not in _cache:
        _cache[k] = _build(k)
    return _cache[k]


def kernel(inputs, k, _trace=False):
    k = int(k)
    x = np.ascontiguousarray(np.asarray(inputs, dtype=np.float32))
    assert x.shape == (N_CORES * ROWS_PER_CORE, N)
    nc = _get(k)
    iota = np.tile(np.arange(WIN, dtype=np.float32), (128, 1))
    in_maps = [
        {"x": x[c * ROWS_PER_CORE:(c + 1) * ROWS_PER_CORE], "iota": iota}
        for c in range(N_CORES)
    ]
    try:
        res = bass_utils.run_bass_kernel_spmd(
            nc, in_maps, core_ids=list(range(N_CORES)), trace=_trace)
    except ModuleNotFoundError:
        res = bass_utils.run_bass_kernel_spmd(
            nc, in_maps, core_ids=list(range(N_CORES)), trace=False)
    out = np.concatenate([r["out"] for r in res.results], axis=0)
    if _trace:
        return out, res
    return out


# revision 8
# speedup vs baseline: 22236.5042x; 22236.5042x over previous
"""KSparse top-k row masking on 8 trn2 NeuronCores.

Algorithm per 128-row tile (rows on partitions, 8192 elements on free dim):
  1. P counting passes on the Scalar engine: activation(Sign, bias=-t) with
     accum_out gives  s = #gt - #le ;  c = (s + n)/2  exactly.  Per-row Newton
     updates with damped fixed density; a row freezes its threshold once its
     count lands in the window [k-31, k].
  2. Extraction on the Vector engine: y = (x is_le t_f) * x keeps the excluded
     elements (values preserved); 64 seg-max8 ops build a candidate array C
     [128, 512] that provably contains the top-32 excluded values; 4 rounds of
     (max8 + match_replace) yield G [128, 32] = sorted top-32 excluded.
     v* = G[k - c_f]  ==  (k+1)-th largest of the row, bit-exact.
  3. Apply: out = (x is_gt v*) * x   (bit-identical to the reference mask-mul).

Data parallel across 8 cores on the batch axis (512 rows/core), no comms.
"""
import math
import sys

sys.path.insert(0, "/opt/trn_rl_repo")

import numpy as np

import concourse.bacc as bacc
import concourse.mybir as mybir
import concourse.tile as tile
from concourse import bass_utils

dt = mybir.dt
Alu = mybir.AluOpType
Act = mybir.ActivationFunctionType

N_CORES = 8
ROWS_PER_CORE = 512
N = 8192
N_TILES = ROWS_PER_CORE // 128

DAMPS = (1.0, 0.8, 0.6)   # P = len+1 counting passes (sim: 0 window-misses
                          # on key(0) and 8 random 4096-row sets)
WIN = 32                         # extraction window (count in [k-31, k])
SEG = 64                         # segments for seg-max8 (128 elements each)


def _norm_ppf(p):
    """Inverse standard normal CDF via bisection on erfc (no scipy)."""
    lo, hi = -10.0, 10.0
    for _ in range(200):
        mid = 0.5 * (lo + hi)
        if 0.5 * math.erfc(-mid / math.sqrt(2.0)) < p:
            lo = mid
        else:
            hi = mid
    return 0.5 * (lo + hi)


def _build(k):
    n = N
    t_target = k - (WIN - 1) / 2.0                  # aim count at window center
    t0 = _norm_ppf(1.0 - t_target / n)
    dens = n * math.exp(-0.5 * t0 * t0) / math.sqrt(2.0 * math.pi)
    c_off = n / 2.0 - t_target                      # d = s*0.5 + c_off  (= c - T)
    r_off = (WIN - 1) / 2.0                         # r = k - c = r_off - d
    half_win = WIN / 2.0 - 0.25                     # |d| <= 15.5 (strict margin)

    nc = bacc.Bacc("TRN2", target_bir_lowering=False, debug=False,
                   num_devices=N_CORES)
    x_dram = nc.dram_tensor("x", [ROWS_PER_CORE, N], dt.float32,
                            kind="ExternalInput").ap()
    iota_dram = nc.dram_tensor("iota", [128, WIN], dt.float32,
                               kind="ExternalInput").ap()
    out_dram = nc.dram_tensor("out", [ROWS_PER_CORE, N], dt.float32,
                              kind="ExternalOutput").ap()

    with tile.TileContext(nc) as tc:
        with (
            tc.tile_pool(name="xp", bufs=N_TILES) as xp,
            tc.tile_pool(name="yp", bufs=2) as yp,
            tc.tile_pool(name="scr", bufs=1) as scrp,
            tc.tile_pool(name="small", bufs=1) as sp,
            tc.tile_pool(name="cand", bufs=2) as cp,
        ):
            iota32 = sp.tile([128, WIN], dt.float32)
            nc.sync.dma_start(iota32[:], iota_dram[:])

            sgn_scratch = scrp.tile([128, N], dt.float8e4)

            xts = []
            for i in range(N_TILES):
                xt = xp.tile([128, N], dt.float32, tag="x")
                nc.sync.dma_start(xt[:], x_dram[i * 128:(i + 1) * 128, :])
                xts.append(xt)

            for i in range(N_TILES):
                xt = xts[i]
                # --- per-tile iteration state [128,1]
                negt = sp.tile([128, 1], dt.float32, tag=f"negt{i}")
                nb = sp.tile([128, 1], dt.float32, tag=f"nb{i}")
                db = sp.tile([128, 1], dt.float32, tag=f"db{i}")
                hg = sp.tile([128, 1], dt.float32, tag=f"hg{i}")
                nc.vector.memset(negt[:], -t0)
                nc.vector.memset(nb[:], -t0)   # negt_best (fallback = current)
                nc.vector.memset(db[:], 0.0)
                nc.vector.memset(hg[:], 0.0)

                s_acc = sp.tile([128, 1], dt.float32, tag=f"s{i}")
                d = sp.tile([128, 1], dt.float32, tag=f"d{i}")
                w = sp.tile([128, 1], dt.float32, tag=f"w{i}")
                wb = sp.tile([128, 1], dt.float32, tag=f"wb{i}")
                good = sp.tile([128, 1], dt.float32, tag=f"g{i}")
                tmp = sp.tile([128, 1], dt.float32, tag=f"tmp{i}")
                nu = sp.tile([128, 1], dt.float32, tag=f"nu{i}")

                n_pass = len(DAMPS) + 1
                for p in range(n_pass):
                    # count: s = sum sign(x - t)
                    nc.scalar.activation(sgn_scratch[:], xt[:], Act.Sign,
                                         bias=negt[:], scale=1.0,
                                         accum_out=s_acc[:])
                    # d = c - T = s*0.5 + c_off   (two single-scalar ops:
                    # dual-scalar tensor_scalar drops op1 on this path)
                    nc.vector.tensor_scalar(d[:], s_acc[:], 0.5, None, Alu.mult)
                    nc.vector.tensor_scalar(d[:], d[:], c_off, None, Alu.add)
                    # w = (d >= -half_win) * (d <= half_win)
                    nc.vector.tensor_scalar(w[:], d[:], -half_win, None,
                                            Alu.is_ge)
                    nc.vector.tensor_scalar(wb[:], d[:], half_win, None,
                                            Alu.is_le)
                    nc.vector.tensor_tensor(w[:], w[:], wb[:], Alu.mult)
                    # good = w AND NOT hg  (both 0/1)  = (w is_gt hg)
                    nc.vector.tensor_tensor(good[:], w[:], hg[:], Alu.is_gt)
                    # hg |= w
                    nc.vector.tensor_tensor(hg[:], hg[:], w[:], Alu.max)
                    # nb += good*(negt - nb);  db += good*(d - db)
                    nc.vector.tensor_tensor(tmp[:], negt[:], nb[:], Alu.subtract)
                    nc.vector.scalar_tensor_tensor(nb[:], tmp[:], good[:], nb[:],
                                                   Alu.mult, Alu.add)
                    nc.vector.tensor_tensor(tmp[:], d[:], db[:], Alu.subtract)
                    nc.vector.scalar_tensor_tensor(db[:], tmp[:], good[:], db[:],
                                                   Alu.mult, Alu.add)
                    if p < n_pass - 1:
                        # negt += (hg - 1) * d * damp/dens
                        #   (nu = hg - 1 in {-1, 0};  Kp = damp/dens > 0;
                        #    negt' = negt + nu*d*Kp  ==  t' = t + (1-hg)*d*Kp)
                        kp = DAMPS[p] / dens
                        nc.vector.tensor_scalar(nu[:], hg[:], 1.0, None,
                                                Alu.subtract)
                        nc.vector.tensor_scalar(tmp[:], d[:], kp, None, Alu.mult)
                        nc.vector.scalar_tensor_tensor(negt[:], tmp[:], nu[:],
                                                       negt[:], Alu.mult, Alu.add)

                # fallback for never-frozen rows: use last (negt, d)
                nc.vector.tensor_tensor(tmp[:], negt[:], nb[:], Alu.subtract)
                nc.vector.tensor_scalar(nu[:], hg[:], 1.0, None, Alu.subtract)
                # nb += (1-hg)*(negt-nb)  ->  nb -= nu*tmp
                nc.vector.tensor_scalar(tmp[:], tmp[:], -1.0, None, Alu.mult)
                nc.vector.scalar_tensor_tensor(nb[:], tmp[:], nu[:], nb[:],
                                               Alu.mult, Alu.add)
                nc.vector.tensor_tensor(tmp[:], d[:], db[:], Alu.subtract)
                nc.vector.tensor_scalar(tmp[:], tmp[:], -1.0, None, Alu.mult)
                nc.vector.scalar_tensor_tensor(db[:], tmp[:], nu[:], db[:],
                                               Alu.mult, Alu.add)

                tpos = sp.tile([128, 1], dt.float32, tag=f"tp{i}")
                nc.vector.tensor_scalar(tpos[:], nb[:], -1.0, None, Alu.mult)

                # --- extraction
                y = yp.tile([128, N], dt.float32, tag="y")
                nc.vector.scalar_tensor_tensor(y[:], xt[:], tpos[:], xt[:],
                                               Alu.is_le, Alu.mult)
                cand = cp.tile([128, SEG * 8], dt.float32, tag="c")
                segw = N // SEG
                for s in range(SEG):
                    nc.vector.max(out=cand[:, s * 8:(s + 1) * 8],
                                  in_=y[:, s * segw:(s + 1) * segw])
                G = sp.tile([128, WIN], dt.float32, tag=f"G{i}")
                rounds = WIN // 8
                for j in range(rounds):
                    nc.vector.max(out=G[:, j * 8:(j + 1) * 8], in_=cand[:])
                    if j < rounds - 1:
                        cand2 = cp.tile([128, SEG * 8], dt.float32, tag="c")
                        nc.vector.match_replace(out=cand2[:],
                                                in_to_replace=G[:, j * 8:(j + 1) * 8],
                                                in_values=cand[:],
                                                imm_value=-1e30)
                        cand = cand2

                # r = r_off - d_f   in [0, WIN-1], clamped
                r = sp.tile([128, 1], dt.float32, tag=f"r{i}")
                nc.vector.tensor_scalar(r[:], db[:], -1.0, r_off,
                                        Alu.mult, Alu.add)
                nc.vector.tensor_scalar(r[:], r[:], 0.0, float(WIN - 1),
                                        Alu.max, Alu.min)
                oh = sp.tile([128, WIN], dt.float32, tag=f"oh{i}")
                nc.vector.tensor_scalar(oh[:], iota32[:], r[:], None,
                                        Alu.is_equal)
                trash = sp.tile([128, WIN], dt.float32, tag=f"tr{i}")
                vhat = sp.tile([128, 1], dt.float32, tag=f"v{i}")
                nc.vector.scalar_tensor_tensor(trash[:], oh[:], 1.0, G[:],
                                               Alu.mult, Alu.mult,
                                               accum_out=vhat[:])

                # --- apply (write over y slot) + store
                nc.vector.scalar_tensor_tensor(y[:], xt[:], vhat[:], xt[:],
                                               Alu.is_gt, Alu.mult)
                nc.sync.dma_start(out_dram[i * 128:(i + 1) * 128, :], y[:])
    nc.compile()
    return nc


_cache = {}


def _get(k):
    if k not in _cache:
        _cache[k] = _build(k)
    return _cache[k]


def kernel(inputs, k, _trace=False):
    k = int(k)
    x = np.ascontiguousarray(np.asarray(inputs, dtype=np.float32))
    assert x.shape == (N_CORES * ROWS_PER_CORE, N)
    nc = _get(k)
    iota = np.tile(np.arange(WIN, dtype=np.float32), (128, 1))
    in_maps = [
        {"x": x[c * ROWS_PER_CORE:(c + 1) * ROWS_PER_CORE], "iota": iota}
        for c in range(N_CORES)
    ]
    try:
        res = bass_utils.run_bass_kernel_spmd(
            nc, in_maps, core_ids=list(range(N_CORES)), trace=_trace)
    except ModuleNotFoundError:
        res = bass_utils.run_bass_kernel_spmd(
            nc, in_maps, core_ids=list(range(N_CORES)), trace=False)
    out = np.concatenate([r["out"] for r in res.results], axis=0)
    if _trace:
        return out, res
    return out
